# revision 6
# baseline (speedup 1.0000x reference)
"""Trainium2 Bass kernel for CharacterBERT CharCNN (char-CNN + highway + proj).

Self-contained: takes FULL inputs (as produced by the reference's
setup_inputs), shards the 4096 tokens data-parallel across 8 NeuronCores,
runs one SPMD Bass/Tile program per core, and gathers the full output.

Per-core pipeline (512 tokens):
  1. Embedding via rank-16 factorization: emb = coords @ basis (SVD, exact
     since EMB=16). Host ships C [16, NIDX] bf16 (per-position coords) and a
     basis lhsT [16, 128] (8 replicated 16-channel groups); one contraction-16
     matmul per 512 cols -> x_rep [128, NIDX].
  2. 7 SBUF->SBUF "skew" DMAs build x_skew [120 rows, 26000 cols] where row
     16*dw+c at col (n,p) holds emb[ids[n, p+dw]][c] * 256; rows 112..118
     carry per-position filter-width mask indicators, row 119 is bias row.
  3. Conv = single matmul per 128-filter chunk against a combined, width-
     padded weight matrix W_all [120, 2048] scaled x256 (mask rows inject
     -30000 at positions invalid for each filter's width; bias folded).
  4. Fused relu+masked-maxpool via DVE tensor_tensor_scan; token maxes
     extracted at pos 49 with strided copies -> tT [128, 16, 520] bf16
     holding 256*t feature-major.
  5. Two highway layers in fp8 (e4m3) DoubleRow matmuls: weights host-split
     hi/lo (2-term: Whi@x8 + Wlo@x8 with x8 = fp8(256 t)); sigmoid/relu on
     ACT straight from PSUM with descaling; combine on DVE in bf16.
  6. Projection in 3-term fp8 DoubleRow (weights hi/lo AND activations
     hi/lo, dropping the lo*lo term); output feature-major [6,128,512] fp32,
     transposed on host.
"""

import numpy as np
import ml_dtypes
from contextlib import ExitStack

import concourse.bass as bass
import concourse.mybir as mybir
import concourse.tile as tile
from concourse import bacc
from concourse.bass_utils import run_bass_kernel_spmd

BF16 = mybir.dt.bfloat16
F32 = mybir.dt.float32
FP8 = mybir.dt.float8e4
AF = mybir.ActivationFunctionType
DR = mybir.MatmulPerfMode.DoubleRow

# problem geometry (hardcoded)
B, S, MAX_CHARS = 8, 512, 50
EMB = 16
VOCAB = 264
TOTAL_F = 2048
HIDDEN = 768
FILTERS = [(1, 32), (2, 32), (3, 64), (4, 128), (5, 256), (6, 512), (7, 1024)]
NCORES = 8

# per-core geometry
T = 512                  # real tokens per core
TPAD = 520               # padded tokens (multiple of 10 for 50-col grouping)
P50 = MAX_CHARS
COLS = TPAD * P50        # 26000
HCOLS = COLS // 2        # 13000 (260 tokens per half)
NIDX = 26112             # gather indices, %128 == 0, >= COLS + 6
KCONV = 120              # 112 patch rows + 7 mask rows + 1 bias row
NEG = -30000.0
SCAN_G = 1024
NKF = TOTAL_F // 128     # 16 filter chunks
NKH = TOTAL_F // 128     # 16 contraction chunks for highway/proj
NOH = 2 * TOTAL_F // 128  # 32 highway output chunks
NOP = HIDDEN // 128      # 6 proj output chunks

SX = 256.0               # activation scale (conv weights pre-scaled)
SW = 1024.0              # fp8 weight scale

_BF = ml_dtypes.bfloat16
_F8 = ml_dtypes.float8_e4m3fn


def _bf(x):
    return np.asarray(x, dtype=np.float32).astype(_BF)


def build_program(ExitStackCls=ExitStack):
    """Build + compile the single-core SPMD Bass program. Returns nc."""
    nc = bacc.Bacc("TRN2", target_bir_lowering=False, debug=False)

    d_cm = nc.dram_tensor("cm", [EMB, NIDX], BF16, kind="ExternalInput").ap()
    d_er = nc.dram_tensor("er", [EMB, 128], BF16, kind="ExternalInput").ap()
    d_pat = nc.dram_tensor("pat", [8, COLS], BF16, kind="ExternalInput").ap()
    d_mmul = nc.dram_tensor("mmul", [128, SCAN_G + P50 - 1], BF16, kind="ExternalInput").ap()
    d_wall = nc.dram_tensor("wall", [KCONV, TOTAL_F], BF16, kind="ExternalInput").ap()
    d_hw0w = nc.dram_tensor("hw0w", [NOH, 128, 32 * 128], FP8, kind="ExternalInput").ap()
    d_hw1w = nc.dram_tensor("hw1w", [NOH, 128, 32 * 128], FP8, kind="ExternalInput").ap()
    d_prjw = nc.dram_tensor("prjw", [NOP, 128, 48 * 128], FP8, kind="ExternalInput").ap()
    d_hwb = nc.dram_tensor("hwb", [128, 64], F32, kind="ExternalInput").ap()
    d_prjb = nc.dram_tensor("prjb", [128, NOP], F32, kind="ExternalInput").ap()
    d_out = nc.dram_tensor("out", [NOP, 128, T], F32, kind="ExternalOutput").ap()
    d_hw_w = [d_hw0w, d_hw1w]

    with tile.TileContext(nc) as tc, ExitStackCls() as ctx:
        const = ctx.enter_context(tc.tile_pool(name="const", bufs=1))
        cm_p = ctx.enter_context(tc.tile_pool(name="cmp", bufs=2))
        xrep_p = ctx.enter_context(tc.tile_pool(name="xrep", bufs=3))
        xskew_p = ctx.enter_context(tc.tile_pool(name="xskew", bufs=1))
        tmaj = ctx.enter_context(tc.tile_pool(name="tmaj", bufs=2))
        x8_p = ctx.enter_context(tc.tile_pool(name="x8p", bufs=2))
        scano_p = ctx.enter_context(tc.tile_pool(name="scano", bufs=3))
        hww_p = ctx.enter_context(tc.tile_pool(name="hww", bufs=3))
        hwtmp = ctx.enter_context(tc.tile_pool(name="hwtmp", bufs=2))
        outp = ctx.enter_context(tc.tile_pool(name="outp", bufs=2))
        convps = ctx.enter_context(tc.tile_pool(name="convps", bufs=2, space="PSUM"))
        nlps = ctx.enter_context(tc.tile_pool(name="nlps", bufs=2, space="PSUM"))
        gps = ctx.enter_context(tc.tile_pool(name="gps", bufs=2, space="PSUM"))

        # ---- constants ----
        er_t = const.tile([EMB, 128], BF16)
        nc.sync.dma_start(er_t[:], d_er[:])
        mm_t = const.tile([128, SCAN_G + P50 - 1], BF16)
        nc.sync.dma_start(mm_t[:], d_mmul[:])
        wall_t = const.tile([KCONV, TOTAL_F], BF16)
        nc.sync.dma_start(wall_t[:], d_wall[:])
        hwb_t = const.tile([128, 64], F32)
        nc.sync.dma_start(hwb_t[:], d_hwb[:])
        prjb_t = const.tile([128, NOP], F32)
        nc.sync.dma_start(prjb_t[:], d_prjb[:])

        # ---- embedding via rank-16 coords matmul, streamed in column chunks ----
        x_skew = xskew_p.tile([KCONV, COLS], BF16)
        nc.sync.dma_start(out=x_skew[112:120, :], in_=d_pat[:, :])
        GCH = 2048
        for c0 in range(0, NIDX, GCH):
            n = min(GCH, NIDX - c0)
            cmt = cm_p.tile([EMB, GCH], BF16, tag="cm")
            nc.sync.dma_start(cmt[:, :n], d_cm[:, c0:c0 + n])
            xr = xrep_p.tile([128, GCH], BF16, tag="xr")
            for b in range(0, n, 512):
                ps = gps.tile([128, 512], F32, tag="gps")
                nc.tensor.matmul(ps[:, :], lhsT=er_t[:, :], rhs=cmt[:, b:b + 512],
                                 start=True, stop=True)
                nc.scalar.copy(xr[:, b:b + 512], ps[:, :])
            # skew copies out of this chunk
            for g in range(7):
                lo = max(0, c0 - g)
                hi = min(c0 + n - g, COLS)
                if hi > lo:
                    nc.sync.dma_start(
                        out=x_skew[16 * g:16 * (g + 1), lo:hi],
                        in_=xr[16 * g:16 * (g + 1), lo - (c0 - g):hi - (c0 - g)])

        # ---- conv + scan-maxpool-relu -> tT [128, 16, TPAD] bf16 (256*t) ----
        tT = tmaj.tile([128, NKF, TPAD], BF16, tag="t")
        for h in range(2):
            h0 = h * HCOLS
            for k in range(NKF):
                prev = None
                c0 = 0
                while c0 < HCOLS:
                    n = min(SCAN_G, HCOLS - c0)
                    ps = convps.tile([128, SCAN_G], F32)
                    for b in range(0, n, 512):
                        m = min(512, n - b)
                        nc.tensor.matmul(
                            ps[:, b:b + m],
                            lhsT=wall_t[:, 128 * k:128 * (k + 1)],
                            rhs=x_skew[:, h0 + c0 + b:h0 + c0 + b + m],
                            start=True, stop=True,
                        )
                    so = scano_p.tile([128, SCAN_G], BF16)
                    ph = c0 % P50
                    nc.vector.tensor_tensor_scan(
                        out=so[:, :n],
                        data0=mm_t[:, ph:ph + n],
                        data1=ps[:, :n],
                        initial=(0.0 if prev is None else prev),
                        op0=mybir.AluOpType.mult,
                        op1=mybir.AluOpType.max,
                    )
                    prev = so[:, n - 1:n]
                    first = (P50 - 1 - c0) % P50
                    if first < n:
                        cnt = (n - first + P50 - 1) // P50
                        tok0 = (h0 + c0 + first) // P50
                        src = (so[:, first:first + P50 * (cnt - 1) + 1:P50]
                               if cnt > 1 else so[:, first:first + 1])
                        nc.vector.tensor_copy(tT[:, k, tok0:tok0 + cnt], src)
                    c0 += n

        # ---- highway layers: 2-term fp8 DoubleRow (W hi/lo, x plain) ----
        knl = 1.0 / SW
        kg = 1.0 / (SX * SW)
        t_in = tT
        for layer in range(2):
            x8 = x8_p.tile([128, NKH, TPAD], FP8, tag="x8", bufs=2)
            nc.vector.tensor_copy(x8[:, :, 0:T], t_in[:, :, 0:T])
            t_out = tmaj.tile([128, NKF, TPAD], BF16, tag="t")
            for j in range(NKH):
                w_nl = hww_p.tile([128, 32, 128], FP8, tag="w_nl", bufs=2)
                nc.sync.dma_start(w_nl[:], d_hw_w[layer][j, :, :])
                w_g = hww_p.tile([128, 32, 128], FP8, tag="w_g", bufs=2)
                nc.sync.dma_start(w_g[:], d_hw_w[layer][j + 16, :, :])
                b_nl = hwb_t[:, layer * 32 + j:layer * 32 + j + 1]
                b_g = hwb_t[:, layer * 32 + 16 + j:layer * 32 + 16 + j + 1]
                ps_nl = nlps.tile([128, T], F32, tag="hwps")
                ps_g = gps.tile([128, T], F32, tag="gps")
                for ps, w8 in ((ps_nl, w_nl), (ps_g, w_g)):
                    for g2 in range(8):
                        rhs = x8[:, 2 * g2:2 * g2 + 2, 0:T]
                        nc.tensor.matmul(
                            ps[:, :], lhsT=w8[:, 4 * g2:4 * g2 + 2, :], rhs=rhs,
                            start=(g2 == 0), stop=False, perf_mode=DR)
                        nc.tensor.matmul(
                            ps[:, :], lhsT=w8[:, 4 * g2 + 2:4 * g2 + 4, :], rhs=rhs,
                            start=False, stop=(g2 == 7), perf_mode=DR)
                sg = hwtmp.tile([128, T], BF16, tag="sg")
                nc.scalar.activation(sg[:, :], ps_g[:, :], AF.Sigmoid, bias=b_g, scale=kg)
                rl = hwtmp.tile([128, T], BF16, tag="rl")
                nc.scalar.activation(rl[:, :], ps_nl[:, :], AF.Relu, bias=b_nl, scale=knl)
                dd = hwtmp.tile([128, T], BF16, tag="dd")
                nc.vector.tensor_sub(dd[:, :], t_in[:, j, 0:T], rl[:, :])
                ee = hwtmp.tile([128, T], BF16, tag="ee")
                nc.vector.tensor_mul(ee[:, :], sg[:, :], dd[:, :])
                nc.vector.tensor_add(t_out[:, j, 0:T], ee[:, :], rl[:, :])
            t_in = t_out

        # ---- projection: 3-term fp8 DoubleRow ----
        xp = x8_p.tile([128, 2 * NKH, TPAD], FP8, tag="xp", bufs=1)
        nc.vector.tensor_copy(xp[:, 0:2 * NKH:2, 0:T], t_in[:, :, 0:T])
        nc.vector.scalar_tensor_tensor(
            out=xp[:, 1:2 * NKH:2, 0:T],
            in0=xp[:, 0:2 * NKH:2, 0:T], scalar=-1.0, in1=t_in[:, :, 0:T],
            op0=mybir.AluOpType.mult, op1=mybir.AluOpType.add)
        kp = 1.0 / (SX * SW)
        for o in range(NOP):
            w_p = hww_p.tile([128, 48, 128], FP8, tag="wp", bufs=2)
            nc.sync.dma_start(w_p[:], d_prjw[o, :, :])
            ps = nlps.tile([128, T], F32, tag="hwps")
            for g2 in range(8):
                nc.tensor.matmul(
                    ps[:, :], lhsT=w_p[:, 6 * g2:6 * g2 + 2, :],
                    rhs=xp[:, 4 * g2:4 * g2 + 2, 0:T],
                    start=(g2 == 0), stop=False, perf_mode=DR)
                nc.tensor.matmul(
                    ps[:, :], lhsT=w_p[:, 6 * g2 + 2:6 * g2 + 4, :],
                    rhs=xp[:, 4 * g2:4 * g2 + 4:2, 0:T],
                    start=False, stop=False, perf_mode=DR)
                nc.tensor.matmul(
                    ps[:, :], lhsT=w_p[:, 6 * g2 + 4:6 * g2 + 6, :],
                    rhs=xp[:, 4 * g2 + 2:4 * g2 + 4, 0:T],
                    start=False, stop=(g2 == 7), perf_mode=DR)
            ot = outp.tile([128, T], F32)
            nc.scalar.activation(ot[:, :], ps[:, :], AF.Identity,
                                 bias=prjb_t[:, o:o + 1], scale=kp)
            nc.sync.dma_start(out=d_out[o, :, :], in_=ot[:, :])

    nc.compile()
    return nc


# ---------------- host-side preparation ----------------

def _fp8_split(x):
    hi = np.asarray(x, np.float32).astype(_F8)
    lo = (np.asarray(x, np.float32) - hi.astype(np.float32)).astype(_F8)
    return hi, lo


def prep_shared(char_emb, conv_ws, conv_bs, hw_ws, hw_bs, proj_w, proj_b):
    """Host repack of all parameters (shared across cores)."""
    out = {}
    # rank-16 factorization of the bf16 embedding table
    embf = _bf(char_emb).astype(np.float32)           # [264, 16]
    U, Sv, Vt = np.linalg.svd(embf, full_matrices=False)
    coords = (U * Sv[None, :]).astype(np.float32)     # [264, 16]
    out["_coords"] = _bf(coords)                      # host-side only
    er = np.zeros((EMB, 128), dtype=_BF)
    for g in range(8):
        er[:, 16 * g:16 * (g + 1)] = _bf(Vt)
    out["er"] = er

    # pattern rows: j-indicator (rows 0..6) period 50, ones row (row 7)
    pat = np.zeros((8, COLS), dtype=_BF)
    pos = np.arange(COLS) % P50
    for j in range(7):
        pat[j] = (pos >= P50 - j).astype(_BF)
    pat[7] = 1.0
    out["pat"] = pat

    # scan multiplier mask, periodic phase tile
    mpos = np.arange(SCAN_G + P50 - 1) % P50
    out["mmul"] = np.tile((mpos != 0).astype(_BF), (128, 1))

    # combined conv weight [120, 2048], scaled x SX
    wall = np.zeros((KCONV, TOTAL_F), dtype=np.float32)
    fbase = 0
    for (w, nf), cw, cb in zip(FILTERS, conv_ws, conv_bs):
        cw = np.asarray(cw, np.float32)  # [nf, 16, w]
        for dw in range(w):
            wall[16 * dw:16 * (dw + 1), fbase:fbase + nf] = SX * cw[:, :, dw].T
        wall[112 + (w - 1), fbase:fbase + nf] = NEG if w > 1 else 0.0
        wall[119, fbase:fbase + nf] = SX * np.asarray(cb, np.float32)
        fbase += nf
    out["wall"] = wall.astype(_BF)

    # highway weights: per out-chunk, 32 fp8 slots [hh(2i),hh(2i+1),lh(2i),lh(2i+1)]
    def repack_hw(wm):
        wm = SW * np.asarray(wm, np.float32)          # [2048, 4096]
        slots = np.zeros((NOH, 32, 128, 128), dtype=_F8)  # [o, slot, k, m]
        for o in range(NOH):
            for kc in range(NKH):
                ws = wm[128 * kc:128 * (kc + 1), 128 * o:128 * (o + 1)]
                hi, lo = _fp8_split(ws)
                base = 4 * (kc // 2) + (kc % 2)
                slots[o, base] = hi
                slots[o, base + 2] = lo
        return slots.transpose(0, 2, 1, 3).reshape(NOH, 128, 32 * 128)

    out["hw0w"] = repack_hw(hw_ws[0])
    out["hw1w"] = repack_hw(hw_ws[1])

    # proj weights: per out-chunk, 48 slots per 2kc: [hh(k),hh(k),lh(k),hh(k'),lh(k'),hh(k')]
    wm = SW * np.asarray(proj_w, np.float32)          # [2048, 768]
    slots = np.zeros((NOP, 48, 128, 128), dtype=_F8)
    for o in range(NOP):
        for k2 in range(8):
            k, kp_ = 2 * k2, 2 * k2 + 1
            hi_k, lo_k = _fp8_split(wm[128 * k:128 * (k + 1), 128 * o:128 * (o + 1)])
            hi_kp, lo_kp = _fp8_split(wm[128 * kp_:128 * (kp_ + 1), 128 * o:128 * (o + 1)])
            sl = slots[o]
            sl[6 * k2 + 0] = hi_k
            sl[6 * k2 + 1] = hi_k
            sl[6 * k2 + 2] = lo_k
            sl[6 * k2 + 3] = hi_kp
            sl[6 * k2 + 4] = lo_kp
            sl[6 * k2 + 5] = hi_kp
    out["prjw"] = slots.transpose(0, 2, 1, 3).reshape(NOP, 128, 48 * 128)

    # biases: hwb [128, 64]: col layout layer*32 + which*16 + j
    # nl biases pre-scaled x SX (output stays in 256-domain); gate biases exact
    hwb = np.zeros((128, 64), dtype=np.float32)
    for layer in range(2):
        hb = np.asarray(hw_bs[layer], np.float32)
        for j in range(16):
            hwb[:, layer * 32 + j] = SX * hb[128 * j:128 * (j + 1)]
            hwb[:, layer * 32 + 16 + j] = hb[TOTAL_F + 128 * j:TOTAL_F + 128 * (j + 1)]
    out["hwb"] = hwb
    out["prjb"] = np.asarray(proj_b, np.float32).reshape(NOP, 128).T.copy()
    return out


def prep_cm(ids_core, coords_bf):
    """ids_core [T, 50] int -> C [16, NIDX] bf16 of per-position coords."""
    flat = ids_core.reshape(-1).astype(np.int64)
    cm = np.zeros((EMB, NIDX), dtype=_BF)
    cm[:, :T * P50] = coords_bf[flat].T
    return cm


_CACHED_NC = None


def _get_nc():
    global _CACHED_NC
    if _CACHED_NC is None:
        _CACHED_NC = build_program()
    return _CACHED_NC


def make_in_maps(inputs):
    ii = {k: np.asarray(v) for k, v in inputs.items()}
    conv_ws = [ii[f"conv_w{i}"] for i in range(7)]
    conv_bs = [ii[f"conv_b{i}"] for i in range(7)]
    shared = prep_shared(
        ii["char_emb"], conv_ws, conv_bs,
        [ii["hw_w0"], ii["hw_w1"]], [ii["hw_b0"], ii["hw_b1"]],
        ii["proj_w"], ii["proj_b"],
    )
    coords_bf = shared.pop("_coords")
    ids = ii["input_ids"].reshape(-1, MAX_CHARS)  # [4096, 50]
    in_maps = []
    for c in range(NCORES):
        m = dict(shared)
        m["cm"] = prep_cm(ids[c * T:(c + 1) * T], coords_bf)
        in_maps.append(m)
    return in_maps


def run(inputs, trace=False, **kw):
    """Run on 8 cores; returns (full_output, BassKernelResults)."""
    in_maps = make_in_maps(inputs)
    res = run_bass_kernel_spmd(_get_nc(), in_maps, list(range(NCORES)),
                               trace=trace, **kw)
    outs = []
    for c in range(NCORES):
        o = np.asarray(res.results[c]["out"])  # [6, 128, T] fp32
        outs.append(o.reshape(HIDDEN, T).T)   # [T, 768]
    full = np.stack(outs, axis=0).reshape(B, S, HIDDEN).astype(np.float32)
    return full, res


def kernel(**inputs):
    return run(inputs)[0]


if __name__ == "__main__":
    # smoke: build only
    build_program()
    print("build ok")
